# revision 7
# baseline (speedup 1.0000x reference)
"""Complex attention layer on 8 Trainium2 NeuronCores.

Sharding: data-parallel over batch (2) x tensor-parallel over heads (16 -> 4
groups of 4). Core c handles batch c//4, heads [4*(c%4), 4*(c%4)+4).
Each core computes its heads' q/k/v projections (f32r matmuls), k-major
softmax (exp on ScalarE, denominators via ones-column matmul), attn @ v, and
its partial output projection. Host sums the per-core output partials (the
"all-reduce"), adds biases, and re-lays-out the attention weights.
"""
import sys

for _p in ("/opt/trn_rl_repo", "/root/.axon_site/_ro/trn_rl_repo"):
    if _p not in sys.path:
        sys.path.append(_p)

import numpy as np
import concourse.bass as bass
import concourse.tile as tile
from concourse import bacc, mybir
from concourse.bass_utils import run_bass_kernel_spmd

B, S, D, H = 2, 2048, 1024, 16
HL = 4                 # heads per core
HD = 64                # head dim
NCORES = 8
SCALE = 1.0 / 8.0
NDT = D // 128         # 8 contraction tiles of 128
NKK = S // 128         # 16 key tiles
QC = 256               # q-chunk width in attention phase
NQC = S // QC          # 8 q-chunks
XC = 256               # s-chunk width in projection phase
NXC = S // XC

F32 = mybir.dt.float32
F32R = mybir.dt.float32r
AF = mybir.ActivationFunctionType

_CACHE = {}


def _build(with_mask: bool):
    nc = bacc.Bacc("TRN2", target_bir_lowering=False)

    xrT = nc.dram_tensor("xrT", [D, S], F32R, kind="ExternalInput")
    xiT = nc.dram_tensor("xiT", [D, S], F32R, kind="ExternalInput")
    wq = nc.dram_tensor("wq", [HL, 128, NDT, HD], F32R, kind="ExternalInput")
    wk = nc.dram_tensor("wk", [HL, 128, NDT, HD], F32R, kind="ExternalInput")
    wv = nc.dram_tensor("wv", [128, NDT, HL * HD], F32R, kind="ExternalInput")
    wo = nc.dram_tensor("wo", [HL, HD, D], F32R, kind="ExternalInput")
    bq = nc.dram_tensor("bq", [HL, HD, 1], F32, kind="ExternalInput")
    bk = nc.dram_tensor("bk", [HL, HD, 1], F32, kind="ExternalInput")
    bv = nc.dram_tensor("bv", [1, 2 * HL * HD], F32R, kind="ExternalInput")
    if with_mask:
        mb = nc.dram_tensor("mb", [1, S], F32R, kind="ExternalInput")
    attn = nc.dram_tensor("attn", [HL, NKK, 128, S], F32, kind="ExternalOutput")
    pout = nc.dram_tensor("pout", [2, S, D], F32, kind="ExternalOutput")

    xrT_v = xrT[:, :].rearrange("(t p) s -> p t s", p=128)
    xiT_v = xiT[:, :].rearrange("(t p) s -> p t s", p=128)

    with tile.TileContext(nc) as tc:
        import contextlib
        ctx = contextlib.ExitStack()
        with ctx:
            const = ctx.enter_context(tc.tile_pool(name="const", bufs=1))
            qkp = ctx.enter_context(tc.tile_pool(name="qkp", bufs=1))
            vp = ctx.enter_context(tc.tile_pool(name="vp", bufs=1))
            dram = ctx.enter_context(tc.tile_pool(name="dram", bufs=1,
                                                  space="DRAM"))

            # ---- constants ----
            wq_t, wk_t, wo_t, bq_t, bk_t = [], [], [], [], []
            for h in range(HL):
                t = const.tile([128, NDT, HD], F32R, name=f"wq{h}")
                nc.sync.dma_start(t, wq[h, :, :, :])
                wq_t.append(t)
                t = const.tile([128, NDT, HD], F32R, name=f"wk{h}")
                nc.sync.dma_start(t, wk[h, :, :, :])
                wk_t.append(t)
                t = const.tile([HD, D], F32R, name=f"wo{h}")
                nc.sync.dma_start(t, wo[h, :, :])
                wo_t.append(t)
                t = const.tile([HD, 1], F32, name=f"bq{h}")
                nc.sync.dma_start(t, bq[h, :, :])
                bq_t.append(t)
                t = const.tile([HD, 1], F32, name=f"bk{h}")
                nc.sync.dma_start(t, bk[h, :, :])
                bk_t.append(t)
            wv_t = const.tile([128, NDT, HL * HD], F32R, name="wvt")
            nc.sync.dma_start(wv_t, wv[:, :, :])
            bv_t = const.tile([1, 2 * HL * HD], F32R, name="bvt")
            nc.sync.dma_start(bv_t, bv[:, :])
            if with_mask:
                mb_t = const.tile([1, S], F32R, name="mbt")
                nc.sync.dma_start(mb_t, mb[:, :])

            ones_f = const.tile([128, 1], F32, name="ones_f")
            nc.vector.memset(ones_f, 1.0)
            ones_col = const.tile([128, 1], F32R, name="ones_col")
            nc.vector.tensor_copy(ones_col, ones_f)
            onesr_f = const.tile([1, 256], F32, name="onesr_f")
            nc.vector.memset(onesr_f, 1.0)
            ones_row = const.tile([1, 256], F32R, name="ones_row")
            nc.vector.tensor_copy(ones_row, onesr_f)

            qcat = [qkp.tile([128, S], F32R, name=f"qcat{h}") for h in range(HL)]
            kcat = [qkp.tile([128, S], F32R, name=f"kcat{h}") for h in range(HL)]
            v_st = [vp.tile([128, HL, 2 * HD], F32R, name=f"vst{k}")
                    for k in range(NKK)]
            out_scr = dram.tile([HL, 2, HD, S], F32R, name="oscr")

            # ---- phase A: projections ----
            with tc.tile_pool(name="xt", bufs=2) as xt, \
                 tc.tile_pool(name="psA", bufs=1, space="PSUM") as psA, \
                 tc.tile_pool(name="tmpA", bufs=4) as tmpA:
                for c in range(NXC):
                    cs = slice(c * XC, (c + 1) * XC)
                    xr_c = xt.tile([128, NDT, XC], F32R, tag="xr", name=f"xr{c}")
                    nc.sync.dma_start(xr_c, xrT_v[:, :, cs])
                    xi_c = xt.tile([128, NDT, XC], F32R, tag="xi", name=f"xi{c}")
                    nc.sync.dma_start(xi_c, xiT_v[:, :, cs])
                    for h in range(HL):
                        pq = psA.tile([HD, XC], F32, tag="pq", name=f"pq{c}{h}")
                        pqi = psA.tile([HD, XC], F32, tag="pqi", name=f"pqi{c}{h}")
                        pk = psA.tile([HD, XC], F32, tag="pk", name=f"pk{c}{h}")
                        pki = psA.tile([HD, XC], F32, tag="pki", name=f"pki{c}{h}")
                        for dt in range(NDT):
                            st, sp = dt == 0, dt == NDT - 1
                            nc.tensor.matmul(pq, wq_t[h][:, dt, :],
                                             xr_c[:, dt, :], start=st, stop=sp)
                            nc.tensor.matmul(pqi, wq_t[h][:, dt, :],
                                             xi_c[:, dt, :], start=st, stop=sp)
                            nc.tensor.matmul(pk, wk_t[h][:, dt, :],
                                             xr_c[:, dt, :], start=st, stop=sp)
                            nc.tensor.matmul(pki, wk_t[h][:, dt, :],
                                             xi_c[:, dt, :], start=st, stop=sp)
                        # aligned halves direct; shifted halves via SBUF DMA
                        nc.scalar.activation(qcat[h][0:HD, cs], pq,
                                             AF.Identity, bias=bq_t[h])
                        tq = tmpA.tile([HD, XC], F32R, tag="tq", name=f"tq{c}{h}")
                        nc.scalar.activation(tq, pqi, AF.Identity, bias=bq_t[h])
                        nc.sync.dma_start(qcat[h][HD:128, cs], tq[0:HD, :])
                        nc.vector.tensor_scalar_add(kcat[h][0:HD, cs], pk,
                                                    bk_t[h])
                        tk = tmpA.tile([HD, XC], F32R, tag="tk", name=f"tk{c}{h}")
                        nc.vector.tensor_scalar_add(tk, pki, bk_t[h])
                        nc.sync.dma_start(kcat[h][HD:128, cs], tk[0:HD, :])
                    # v projection for the two 128-row s-tiles of this chunk
                    for sti in range(XC // 128):
                        kk = c * (XC // 128) + sti
                        ssl = slice(sti * 128, (sti + 1) * 128)
                        pv = psA.tile([128, 2 * HL * HD], F32, tag="pv",
                                      bufs=2, name=f"pv{kk}")
                        nc.tensor.matmul(pv, ones_row[0:1, 0:128],
                                         bv_t, start=True, stop=False)
                        for dt in range(NDT):
                            sp = dt == NDT - 1
                            nc.tensor.matmul(pv[:, 0:HL * HD],
                                             xr_c[:, dt, ssl], wv_t[:, dt, :],
                                             start=False, stop=sp)
                            nc.tensor.matmul(pv[:, HL * HD:2 * HL * HD],
                                             xi_c[:, dt, ssl], wv_t[:, dt, :],
                                             start=False, stop=sp)
                        for h in range(HL):
                            nc.scalar.activation(
                                v_st[kk][:, h, 0:HD],
                                pv[:, h * HD:(h + 1) * HD], AF.Identity)
                            nc.scalar.activation(
                                v_st[kk][:, h, HD:2 * HD],
                                pv[:, HL * HD + h * HD:HL * HD + (h + 1) * HD],
                                AF.Identity)

            # ---- phase B: attention ----
            with tc.tile_pool(name="ptp", bufs=2) as ptp, \
                 tc.tile_pool(name="psB", bufs=2, space="PSUM") as psB, \
                 tc.tile_pool(name="smB", bufs=2) as smB, \
                 tc.tile_pool(name="osg", bufs=2) as osg:
                attn_v = [attn[h].rearrange("t p q -> p t q") for h in range(HL)]
                for h in range(HL):
                    ostage = osg.tile([128, S], F32R, tag="ostage",
                                      name=f"ostage{h}")
                    for qc in range(NQC):
                        qs = slice(qc * QC, (qc + 1) * QC)
                        pt = ptp.tile([128, NKK, QC], F32R, tag="pt",
                                      name=f"pt{h}{qc}")
                        po = psB.tile([128, QC], F32, tag="po", name=f"po{h}{qc}")
                        psum = psB.tile([1, QC], F32, tag="psum",
                                        bufs=1, name=f"psm{h}{qc}")
                        for g4 in range(NKK // 4):
                            ps4 = psB.tile([128, 4, QC], F32, tag="ps4",
                                           name=f"ps{h}{qc}{g4}")
                            for j in range(4):
                                kk = 4 * g4 + j
                                ksl = slice(kk * 128, (kk + 1) * 128)
                                nc.tensor.matmul(ps4[:, j, :], kcat[h][:, ksl],
                                                 qcat[h][:, qs],
                                                 start=True, stop=not with_mask)
                                if with_mask:
                                    nc.tensor.matmul(ps4[:, j, :],
                                                     mb_t[0:1, ksl],
                                                     ones_row[0:1, 0:QC],
                                                     start=False, stop=True)
                            nc.scalar.activation(
                                pt[:, 4 * g4:4 * g4 + 4, :]
                                .rearrange("p a q -> p (a q)"),
                                ps4.rearrange("p a q -> p (a q)"), AF.Exp)
                            for j in range(4):
                                kk = 4 * g4 + j
                                nc.tensor.matmul(psum, ones_col, pt[:, kk, :],
                                                 start=kk == 0,
                                                 stop=kk == NKK - 1)
                                nc.tensor.matmul(po, v_st[kk][:, h, :],
                                                 pt[:, kk, :],
                                                 start=kk == 0,
                                                 stop=kk == NKK - 1)
                        s1 = smB.tile([1, QC], F32, tag="s1", name=f"s1{h}{qc}")
                        nc.vector.tensor_copy(s1, psum)
                        s2 = smB.tile([1, QC], F32, tag="s2", name=f"s2{h}{qc}")
                        nc.vector.reciprocal(s2, s1)
                        s3 = smB.tile([1, QC], F32R, tag="s3", name=f"s3{h}{qc}")
                        nc.vector.tensor_copy(s3, s2)
                        pbc = psB.tile([128, QC], F32, tag="pbc",
                                       bufs=1, name=f"pbc{h}{qc}")
                        nc.tensor.matmul(pbc, ones_row[0:1, 0:128], s3,
                                         start=True, stop=True)
                        bc = smB.tile([128, QC], F32, tag="bc", name=f"bc{h}{qc}")
                        nc.vector.tensor_copy(bc, pbc)
                        nc.vector.tensor_mul(ostage[:, qs], po, bc)
                        for kk in range(NKK):
                            eng = nc.vector if kk % 2 == 0 else nc.gpsimd
                            eng.tensor_mul(pt[:, kk, :], pt[:, kk, :], bc)
                        nc.sync.dma_start(attn_v[h][:, :, qs], pt.bitcast(F32))
                    nc.sync.dma_start(out_scr[h, 0, :, :], ostage[0:HD, :])
                    nc.sync.dma_start(out_scr[h, 1, :, :], ostage[HD:128, :])

            # ---- phase C: output projection ----
            with tc.tile_pool(name="otC", bufs=4) as otC, \
                 tc.tile_pool(name="psC", bufs=2, space="PSUM") as psC, \
                 tc.tile_pool(name="stC", bufs=2) as stC:
                for ri in range(2):
                    ot = []
                    for h in range(HL):
                        t = otC.tile([HD, S], F32R, tag="ot", name=f"ot{ri}{h}")
                        nc.sync.dma_start(t, out_scr[h, ri, :, :])
                        ot.append(t)
                    for st in range(S // 128):
                        ssl = slice(st * 128, (st + 1) * 128)
                        pco = psC.tile([128, D], F32, tag="pco",
                                       name=f"pco{ri}{st}")
                        for ch in range(D // 512):
                            csl = slice(ch * 512, (ch + 1) * 512)
                            for h in range(HL):
                                nc.tensor.matmul(pco[:, csl], ot[h][:, ssl],
                                                 wo_t[h][:, csl],
                                                 start=h == 0, stop=h == HL - 1)
                        stg = stC.tile([128, D], F32, tag="stg",
                                       name=f"stg{ri}{st}")
                        nc.scalar.activation(stg, pco, AF.Identity)
                        nc.scalar.dma_start(pout[ri, ssl, :], stg)
    nc.compile()
    return nc


def kernel(x_real, x_imag, Wq, bq, Wk, bk, Wv, bv, Wo, bo, mask):
    x_real = np.asarray(x_real, dtype=np.float32)
    x_imag = np.asarray(x_imag, dtype=np.float32)
    Wq, bq = np.asarray(Wq, np.float32), np.asarray(bq, np.float32)
    Wk, bk = np.asarray(Wk, np.float32), np.asarray(bk, np.float32)
    Wv, bv = np.asarray(Wv, np.float32), np.asarray(bv, np.float32)
    Wo, bo = np.asarray(Wo, np.float32), np.asarray(bo, np.float32)
    mask = np.asarray(mask)
    with_mask = bool((mask == 0).any())

    key = with_mask
    if key not in _CACHE:
        _CACHE[key] = _build(with_mask)
    nc = _CACHE[key]

    def pack_w(w_rows):  # [64, D] -> [128, NDT, 64]
        return np.ascontiguousarray(
            w_rows.T.reshape(NDT, 128, HD).transpose(1, 0, 2))

    in_maps = []
    for c in range(NCORES):
        b, g = c // 4, c % 4
        im = {
            "xrT": np.ascontiguousarray(x_real[b].T),
            "xiT": np.ascontiguousarray(x_imag[b].T),
            "wq": np.stack([pack_w((SCALE * Wq[(4 * g + h) * HD:
                                               (4 * g + h + 1) * HD]
                                    ).astype(np.float32))
                            for h in range(HL)]),
            "wk": np.stack([pack_w(Wk[(4 * g + h) * HD:(4 * g + h + 1) * HD])
                            for h in range(HL)]),
            "wv": np.ascontiguousarray(
                Wv[4 * g * HD:(4 * g + 4) * HD].T
                .reshape(NDT, 128, HL * HD).transpose(1, 0, 2)),
            "wo": np.stack([np.ascontiguousarray(
                Wo[:, (4 * g + h) * HD:(4 * g + h + 1) * HD].T)
                for h in range(HL)]),
            "bq": (SCALE * np.stack(
                [bq[(4 * g + h) * HD:(4 * g + h + 1) * HD, None]
                 for h in range(HL)])).astype(np.float32),
            "bk": np.stack([bk[(4 * g + h) * HD:(4 * g + h + 1) * HD, None]
                            for h in range(HL)]),
            "bv": np.tile(bv[4 * g * HD:(4 * g + 4) * HD], 2)[None, :].copy(),
        }
        if with_mask:
            im["mb"] = np.where(mask[b] == 0, np.float32(-1e9),
                                np.float32(0.0))[None, :].astype(np.float32)
        in_maps.append(im)

    import os
    trace = bool(int(os.environ.get("KBENCH_TRACE", "0")))
    res = run_bass_kernel_spmd(nc, in_maps, core_ids=list(range(NCORES)),
                               trace=trace)
    kernel.last_exec_time_ns = res.exec_time_ns

    out_r = np.zeros((B, S, D), np.float32)
    out_i = np.zeros((B, S, D), np.float32)
    attn_full = np.empty((B, H, S, S), np.float32)
    for c in range(NCORES):
        b, g = c // 4, c % 4
        r = res.results[c]
        out_r[b] += r["pout"][0]
        out_i[b] += r["pout"][1]
        a = r["attn"].reshape(HL, S, S)  # [h, k, q]
        for h in range(HL):
            attn_full[b, 4 * g + h] = a[h].T
    out_r += bo
    out_i += bo
    return out_r, out_i, attn_full
